# revision 25
# baseline (speedup 1.0000x reference)
"""Spatial-reduction attention (PVT-style) on 8 Trainium2 NeuronCores.

Shapes (hardcoded): x [4, 4096, 512], 8 heads, head_dim 64, SR=2 conv
reduction -> 1024 keys. Sharding: core c handles batch c//2, query half
c%2 (2048 queries). Conv + kv are recomputed per core pair (cheaper than
a cross-core exchange). All matmul operands bf16, fp32 PSUM accumulate.

Per-core dataflow (everything kept transposed, [channel, token]):
  qT   = q_wT.T @ xq            [512, 2048]
  convT= sum_ij srw_ij.T @ gather_ij(xf) + sr_b   [512, 1024]
  kT   = k_wT.T @ convT         [512, 1024]
  v    = convT.T @ v_wT         [1024, 512]  (natural layout, +ones col)
  ST_h = kT_h.T @ qT_h          [1024, 2048] per head (row-tiled pairs)
  E    = exp(ST * scale)        (ScalarE, bf16 out)
  O_h  = v_aug_h.T @ E          [65, 2048]  (row 64 = softmax denom)
  OT   = O_h / denom            [512, 2048] bf16
  PT   = proj_wT.T @ OT + proj_b  [512, 2048] fp32 -> output (host transposes)
"""

import numpy as np
import ml_dtypes
from contextlib import ExitStack

import concourse.bass as bass
import concourse.mybir as mybir
from concourse import bacc
from concourse.bass_utils import run_bass_kernel_spmd
from concourse.tile import TileContext

BF = mybir.dt.bfloat16
F8 = mybir.dt.float8e4
F32 = mybir.dt.float32
P = 128
CT = 4            # channel tiles (512 / 128)
NQ = 2048         # queries per core
NKT = 8           # key tiles (1024 / 128)
SCALE = 0.125     # 64 ** -0.5

_CACHE = {}


def _build_program():
    nc = bacc.Bacc("TRN2", target_bir_lowering=False, debug=False, num_devices=8)

    xq_d = nc.dram_tensor("xq", [512, NQ], BF, kind="ExternalInput")
    xf_d = nc.dram_tensor("xf", [512, 4096], BF, kind="ExternalInput")
    qw_d = nc.dram_tensor("qw", [512, 512], BF, kind="ExternalInput")      # [c, dq]
    kw_d = nc.dram_tensor("kw", [512, 512], BF, kind="ExternalInput")      # [c, dk]
    vw_d = nc.dram_tensor("vw", [512, 512], BF, kind="ExternalInput")      # [c, dv]
    srw_d = nc.dram_tensor("srw", [4, 512, 512], BF, kind="ExternalInput")  # [ij, ci, co]
    srb_d = nc.dram_tensor("srb", [512], F32, kind="ExternalInput")
    pw_d = nc.dram_tensor("pw", [512, 512], BF, kind="ExternalInput")      # [c, co]
    pb_d = nc.dram_tensor("pb", [512], F32, kind="ExternalInput")
    out_d = nc.dram_tensor("out_t", [512, NQ], F32, kind="ExternalOutput")

    Exp = mybir.ActivationFunctionType.Exp

    with TileContext(nc) as tc, ExitStack() as ctx:
        const = ctx.enter_context(tc.tile_pool(name="const", bufs=1))
        expp = ctx.enter_context(tc.tile_pool(name="expp", bufs=3))
        d2p = ctx.enter_context(tc.tile_pool(name="d2p", bufs=1))
        rbp = ctx.enter_context(tc.tile_pool(name="rbp", bufs=1))
        outp = ctx.enter_context(tc.tile_pool(name="outp", bufs=2))

        dma = nc.sync.dma_start

        # ---- load inputs ----
        qw_sb = const.tile([P, CT, 512], BF)
        dma(out=qw_sb, in_=qw_d.rearrange("(t p) n -> p t n", p=P))
        xq_sb = const.tile([P, CT, NQ], BF)
        xq_r = xq_d.rearrange("(t p) n -> p t n", p=P)
        for t in range(CT):
            dma(out=xq_sb[:, t, :], in_=xq_r[:, t, :])
        kw_sb = const.tile([P, CT, 512], BF)
        dma(out=kw_sb, in_=kw_d.rearrange("(t p) n -> p t n", p=P))
        vw_sb = const.tile([P, CT, 512], BF)
        dma(out=vw_sb, in_=vw_d.rearrange("(t p) n -> p t n", p=P))
        srw_sb = const.tile([P, 4, CT, 512], BF)
        srw_r = srw_d.rearrange("i (t p) o -> p i t o", p=P)
        for ij4 in range(4):
            dma(out=srw_sb[:, ij4, :, :], in_=srw_r[:, ij4, :, :])
        srb_sb = const.tile([P, CT], F32)
        dma(out=srb_sb, in_=srb_d.rearrange("(t p) -> p t", p=P))
        pw_sb = const.tile([P, CT, 512], BF)
        dma(out=pw_sb, in_=pw_d.rearrange("(t p) n -> p t n", p=P))
        pb_sb = const.tile([P, CT], F32)
        dma(out=pb_sb, in_=pb_d.rearrange("(t p) -> p t", p=P))

        xf_sb = const.tile([P, CT, 4096], BF)
        xf_r = xf_d.rearrange("(t p) n -> p t n", p=P)
        for t in range(CT):
            dma(out=xf_sb[:, t, :], in_=xf_r[:, t, :])

        qT_sb = const.tile([P, CT, NQ], BF)
        convT_sb = const.tile([P, CT, 1024], BF)
        kTz_sb = const.tile([P, 8, 1024], BF)
        vaug_sb = const.tile([P, NKT, 8, 128], BF)
        oT_sb = const.tile([P, CT, NQ], BF)

        nc.gpsimd.memset(vaug_sb, 0.0)
        nc.gpsimd.memset(vaug_sb[:, :, :, 64:65], 1.0)
        nc.gpsimd.memset(kTz_sb, 0.0)

        with ExitStack() as ps_ctx:
            ps1 = ps_ctx.enter_context(tc.tile_pool(name="ps1", bufs=8, space="PSUM"))

            # ---- phase B: qT = q_wT.T @ xq ----
            for dq in range(CT):
                for nqb in range(4):
                    ps = ps1.tile([P, 512], F32)
                    for c in range(CT):
                        nc.tensor.matmul(
                            ps,
                            qw_sb[:, c, dq * 128:(dq + 1) * 128],
                            xq_sb[:, c, nqb * 512:(nqb + 1) * 512],
                            start=(c == 0), stop=(c == CT - 1),
                        )
                    nc.vector.tensor_copy(
                        qT_sb[:, dq, nqb * 512:(nqb + 1) * 512], ps)

            # ---- phase C: convT (spatial reduction) ----
            for co in range(CT):
                for nkb in range(2):
                    ps = ps1.tile([P, 512], F32)
                    n_mm = 0
                    for ij in range(4):
                        i, j = ij >> 1, ij & 1
                        for ci in range(CT):
                            rhs = xf_sb[:, ci, :].rearrange(
                                "p (a i b j) -> p i j a b", a=32, i=2, b=32, j=2
                            )[:, i, j, nkb * 16:(nkb + 1) * 16, :]
                            nc.tensor.matmul(
                                ps,
                                srw_sb[:, ij, ci, co * 128:(co + 1) * 128],
                                rhs,
                                start=(n_mm == 0), stop=(n_mm == 15),
                            )
                            n_mm += 1
                    nc.vector.tensor_scalar_add(
                        convT_sb[:, co, nkb * 512:(nkb + 1) * 512],
                        ps, srb_sb[:, co:co + 1])

            # ---- phase D: kT = k_wT.T @ convT ----
            for kt in range(CT):
                for nkb in range(2):
                    ps = ps1.tile([P, 512], F32)
                    for c in range(CT):
                        nc.tensor.matmul(
                            ps,
                            kw_sb[:, c, kt * 128:(kt + 1) * 128],
                            convT_sb[:, c, nkb * 512:(nkb + 1) * 512],
                            start=(c == 0), stop=(c == CT - 1),
                        )
                    nc.vector.tensor_copy(
                        kTz_sb[0:64, 2 * kt, nkb * 512:(nkb + 1) * 512],
                        ps[0:64, :])
                    nc.vector.tensor_copy(
                        kTz_sb[64:128, 2 * kt + 1, nkb * 512:(nkb + 1) * 512],
                        ps[64:128, :])

            # ---- phase E: v = convT.T @ v_wT (natural layout + ones col) ----
            for nk in range(NKT):
                ps = ps1.tile([P, 512], F32)
                for c in range(CT):
                    nc.tensor.matmul(
                        ps,
                        convT_sb[:, c, nk * 128:(nk + 1) * 128],
                        vw_sb[:, c, :],
                        start=(c == 0), stop=(c == CT - 1),
                    )
                nc.vector.tensor_copy(
                    vaug_sb[:, nk, :, 0:64],
                    ps.rearrange("p (h e) -> p h e", e=64),
                )

        # ---- phase F: attention per head pair, per query half ----
        with ExitStack() as ps_ctx:
            ps_s = ps_ctx.enter_context(
                tc.tile_pool(name="ps_s", bufs=1, space="PSUM"))
            ps_o = ps_ctx.enter_context(
                tc.tile_pool(name="ps_o", bufs=1, space="PSUM"))

            for hf in range(2):
                for pr in range(4):
                    # o[h2][q2]: [65, 512] accumulators (1 PSUM bank each)
                    o_ps = [[ps_o.tile([P, 512], F32, tag=f"o{h2}{q2}",
                                       name=f"o_{pr}_{hf}_{h2}{q2}")
                             for q2 in range(2)] for h2 in range(2)]
                    for nk in range(NKT):
                        s0 = ps_s.tile([P, 1024], F32, tag="s0")
                        s1 = ps_s.tile([P, 1024], F32, tag="s1")
                        for q2 in range(2):
                            nqs = hf * 1024 + q2 * 512
                            nc.tensor.matmul(
                                s0[:, q2 * 512:(q2 + 1) * 512],
                                kTz_sb[:, 2 * pr, nk * 128:(nk + 1) * 128],
                                qT_sb[:, pr, nqs:nqs + 512],
                                start=True, stop=True,
                            )
                            nc.tensor.matmul(
                                s1[:, q2 * 512:(q2 + 1) * 512],
                                kTz_sb[:, 2 * pr + 1, nk * 128:(nk + 1) * 128],
                                qT_sb[:, pr, nqs:nqs + 512],
                                start=True, stop=True,
                            )
                        e0 = expp.tile([P, 1024], BF)
                        e1 = expp.tile([P, 1024], BF)
                        nc.scalar.activation(e0, s0, Exp, scale=SCALE)
                        nc.scalar.activation(e1, s1, Exp, scale=SCALE)
                        for q2 in range(2):
                            qs = q2 * 512
                            nc.tensor.matmul(
                                o_ps[0][q2],
                                vaug_sb[:, nk, 2 * pr, :],
                                e0[:, qs:qs + 512],
                                start=(nk == 0), stop=(nk == NKT - 1),
                            )
                            nc.tensor.matmul(
                                o_ps[1][q2],
                                vaug_sb[:, nk, 2 * pr + 1, :],
                                e1[:, qs:qs + 512],
                                start=(nk == 0), stop=(nk == NKT - 1),
                            )
                    # normalize: OT = O / denom (denom = row 64 of o_ps).
                    # Stage denom rows into SBUF, reciprocal, then per-chunk
                    # broadcast + multiply so each o_ps bank frees asap.
                    d2 = d2p.tile([1, 2048], F32)
                    for h2 in range(2):
                        for q2 in range(2):
                            nc.vector.tensor_copy(
                                d2[0:1, h2 * 1024 + q2 * 512:
                                   h2 * 1024 + (q2 + 1) * 512],
                                o_ps[h2][q2][64:65, :])
                    r2 = d2p.tile([1, 2048], F32)
                    nc.vector.reciprocal_approx_fast(out=r2, in_=d2)
                    for h2 in range(2):
                        for q2 in range(2):
                            rb = rbp.tile([64, 512], F32, tag=f"rb{h2}{q2}",
                                          name=f"rb_{pr}_{hf}_{h2}{q2}")
                            nc.gpsimd.partition_broadcast(
                                rb, r2[0:1, h2 * 1024 + q2 * 512:
                                       h2 * 1024 + (q2 + 1) * 512])
                            hq = hf * 1024 + q2 * 512
                            nc.vector.tensor_mul(
                                oT_sb[h2 * 64:(h2 + 1) * 64, pr, hq:hq + 512],
                                o_ps[h2][q2][0:64, :], rb)


        # ---- phase G: PT = proj_wT.T @ OT + proj_b ----
        with ExitStack() as ps_ctx:
            ps2 = ps_ctx.enter_context(
                tc.tile_pool(name="ps2", bufs=8, space="PSUM"))
            for co in range(CT):
                for nqb in range(4):
                    ps = ps2.tile([P, 512], F32)
                    for c in range(CT):
                        nc.tensor.matmul(
                            ps,
                            pw_sb[:, c, co * 128:(co + 1) * 128],
                            oT_sb[:, c, nqb * 512:(nqb + 1) * 512],
                            start=(c == 0), stop=(c == CT - 1),
                        )
                    pt = outp.tile([P, 512], F32)
                    nc.vector.tensor_scalar_add(pt, ps, pb_sb[:, co:co + 1])
                    dma(out=out_d[co * 128:(co + 1) * 128,
                                  nqb * 512:(nqb + 1) * 512], in_=pt)

    nc.compile()
    return nc


def kernel(x, q_w, kv_w, sr_w, sr_b, proj_w, proj_b, H=64, W=64, **_kw):
    x = np.asarray(x, dtype=np.float32)
    q_w = np.asarray(q_w, dtype=np.float32)
    kv_w = np.asarray(kv_w, dtype=np.float32)
    sr_w = np.asarray(sr_w, dtype=np.float32)
    sr_b = np.asarray(sr_b, dtype=np.float32)
    proj_w = np.asarray(proj_w, dtype=np.float32)
    proj_b = np.asarray(proj_b, dtype=np.float32)
    B, N, C = x.shape

    if "nc" not in _CACHE:
        _CACHE["nc"] = _build_program()
    nc = _CACHE["nc"]

    bf = ml_dtypes.bfloat16
    qw_t = np.ascontiguousarray(q_w.T).astype(bf)              # [c, dq]
    kw_t = np.ascontiguousarray(kv_w[:512].T).astype(bf)       # [c, dk]
    vw_t = np.ascontiguousarray(kv_w[512:].T).astype(bf)       # [c, dv]
    srw_t = np.ascontiguousarray(
        sr_w.transpose(2, 3, 1, 0).reshape(4, 512, 512)).astype(bf)
    pw_t = np.ascontiguousarray(proj_w.T).astype(bf)           # [c, co]

    in_maps = []
    xT = np.ascontiguousarray(x.transpose(0, 2, 1)).astype(bf)  # [B, C, N]
    for c in range(8):
        b, hf = c // 2, c % 2
        in_maps.append({
            "xq": np.ascontiguousarray(xT[b][:, hf * NQ:(hf + 1) * NQ]),
            "xf": xT[b],
            "qw": qw_t, "kw": kw_t, "vw": vw_t,
            "srw": srw_t, "srb": sr_b,
            "pw": pw_t, "pb": proj_b,
        })

    res = run_bass_kernel_spmd(nc, in_maps, core_ids=list(range(8)))
    _CACHE["last_exec_time_ns"] = res.exec_time_ns

    out = np.empty((B, N, C), dtype=np.float32)
    for c in range(8):
        b, hf = c // 2, c % 2
        out[b, hf * NQ:(hf + 1) * NQ, :] = res.results[c]["out_t"].T
    return out


# revision 26
# speedup vs baseline: 1.0169x; 1.0169x over previous
"""Spatial-reduction attention (PVT-style) on 8 Trainium2 NeuronCores.

Shapes (hardcoded): x [4, 4096, 512], 8 heads, head_dim 64, SR=2 conv
reduction -> 1024 keys. Sharding: core c handles batch c//2, query half
c%2 (2048 queries). Conv + kv are recomputed per core pair (cheaper than
a cross-core exchange). All matmul operands bf16, fp32 PSUM accumulate.

Per-core dataflow (everything kept transposed, [channel, token]):
  qT   = q_wT.T @ xq            [512, 2048]
  convT= sum_ij srw_ij.T @ gather_ij(xf) + sr_b   [512, 1024]
  kT   = k_wT.T @ convT         [512, 1024]
  v    = convT.T @ v_wT         [1024, 512]  (natural layout, +ones col)
  ST_h = kT_h.T @ qT_h          [1024, 2048] per head (row-tiled pairs)
  E    = exp(ST * scale)        (ScalarE, bf16 out)
  O_h  = v_aug_h.T @ E          [65, 2048]  (row 64 = softmax denom)
  OT   = O_h / denom            [512, 2048] bf16
  PT   = proj_wT.T @ OT + proj_b  [512, 2048] fp32 -> output (host transposes)
"""

import numpy as np
import ml_dtypes
from contextlib import ExitStack

import concourse.bass as bass
import concourse.mybir as mybir
from concourse import bacc
from concourse.bass_utils import run_bass_kernel_spmd
from concourse.tile import TileContext

BF = mybir.dt.bfloat16
F8 = mybir.dt.float8e4
F32 = mybir.dt.float32
P = 128
CT = 4            # channel tiles (512 / 128)
NQ = 2048         # queries per core
NKT = 8           # key tiles (1024 / 128)
SCALE = 0.125     # 64 ** -0.5

_CACHE = {}


def _build_program():
    nc = bacc.Bacc("TRN2", target_bir_lowering=False, debug=False, num_devices=8)

    xq_d = nc.dram_tensor("xq", [512, NQ], BF, kind="ExternalInput")
    xf_d = nc.dram_tensor("xf", [512, 4096], BF, kind="ExternalInput")
    qw_d = nc.dram_tensor("qw", [512, 512], BF, kind="ExternalInput")      # [c, dq]
    kw_d = nc.dram_tensor("kw", [512, 512], BF, kind="ExternalInput")      # [c, dk]
    vw_d = nc.dram_tensor("vw", [512, 512], BF, kind="ExternalInput")      # [c, dv]
    srw_d = nc.dram_tensor("srw", [4, 512, 512], BF, kind="ExternalInput")  # [ij, ci, co]
    srb_d = nc.dram_tensor("srb", [512], F32, kind="ExternalInput")
    pw_d = nc.dram_tensor("pw", [512, 512], BF, kind="ExternalInput")      # [c, co]
    pb_d = nc.dram_tensor("pb", [512], F32, kind="ExternalInput")
    out_d = nc.dram_tensor("out_t", [512, NQ], F32, kind="ExternalOutput")

    Exp = mybir.ActivationFunctionType.Exp

    with TileContext(nc) as tc, ExitStack() as ctx:
        const = ctx.enter_context(tc.tile_pool(name="const", bufs=1))
        expp = ctx.enter_context(tc.tile_pool(name="expp", bufs=3))
        d2p = ctx.enter_context(tc.tile_pool(name="d2p", bufs=1))
        rbp = ctx.enter_context(tc.tile_pool(name="rbp", bufs=1))
        outp = ctx.enter_context(tc.tile_pool(name="outp", bufs=2))

        dma = nc.sync.dma_start

        # ---- load inputs ----
        qw_sb = const.tile([P, CT, 512], BF)
        dma(out=qw_sb, in_=qw_d.rearrange("(t p) n -> p t n", p=P))
        xq_sb = const.tile([P, CT, NQ], BF)
        xq_r = xq_d.rearrange("(t p) n -> p t n", p=P)
        for t in range(CT):
            dma(out=xq_sb[:, t, :], in_=xq_r[:, t, :])
        kw_sb = const.tile([P, CT, 512], BF)
        dma(out=kw_sb, in_=kw_d.rearrange("(t p) n -> p t n", p=P))
        vw_sb = const.tile([P, CT, 512], BF)
        dma(out=vw_sb, in_=vw_d.rearrange("(t p) n -> p t n", p=P))
        srw_sb = const.tile([P, 4, CT, 512], BF)
        srw_r = srw_d.rearrange("i (t p) o -> p i t o", p=P)
        for ij4 in range(4):
            dma(out=srw_sb[:, ij4, :, :], in_=srw_r[:, ij4, :, :])
        srb_sb = const.tile([P, CT], F32)
        dma(out=srb_sb, in_=srb_d.rearrange("(t p) -> p t", p=P))
        pw_sb = const.tile([P, CT, 512], BF)
        dma(out=pw_sb, in_=pw_d.rearrange("(t p) n -> p t n", p=P))
        pb_sb = const.tile([P, CT], F32)
        dma(out=pb_sb, in_=pb_d.rearrange("(t p) -> p t", p=P))

        xf_sb = const.tile([P, CT, 4096], BF)
        xf_r = xf_d.rearrange("(t p) n -> p t n", p=P)
        for t in range(CT):
            dma(out=xf_sb[:, t, :], in_=xf_r[:, t, :])

        qT_sb = const.tile([P, CT, NQ], BF)
        convT_sb = const.tile([P, CT, 1024], BF)
        kTz_sb = const.tile([P, 8, 1024], BF)
        vaug_sb = const.tile([P, NKT, 8, 128], BF)
        oT_sb = const.tile([P, CT, NQ], BF)

        nc.gpsimd.memset(vaug_sb, 0.0)
        nc.gpsimd.memset(vaug_sb[:, :, :, 64:65], 1.0)
        nc.gpsimd.memset(kTz_sb, 0.0)

        with ExitStack() as ps_ctx:
            ps1 = ps_ctx.enter_context(tc.tile_pool(name="ps1", bufs=6, space="PSUM"))

            # ---- phase B: qT = q_wT.T @ xq ----
            for dq in range(CT):
                for nqb in range(4):
                    ps = ps1.tile([P, 512], F32)
                    for c in range(CT):
                        nc.tensor.matmul(
                            ps,
                            qw_sb[:, c, dq * 128:(dq + 1) * 128],
                            xq_sb[:, c, nqb * 512:(nqb + 1) * 512],
                            start=(c == 0), stop=(c == CT - 1),
                        )
                    nc.vector.tensor_copy(
                        qT_sb[:, dq, nqb * 512:(nqb + 1) * 512], ps)

            # ---- phase C: convT (spatial reduction) ----
            for co in range(CT):
                for nkb in range(2):
                    ps = ps1.tile([P, 512], F32)
                    n_mm = 0
                    for ij in range(4):
                        i, j = ij >> 1, ij & 1
                        for ci in range(CT):
                            rhs = xf_sb[:, ci, :].rearrange(
                                "p (a i b j) -> p i j a b", a=32, i=2, b=32, j=2
                            )[:, i, j, nkb * 16:(nkb + 1) * 16, :]
                            nc.tensor.matmul(
                                ps,
                                srw_sb[:, ij, ci, co * 128:(co + 1) * 128],
                                rhs,
                                start=(n_mm == 0), stop=(n_mm == 15),
                            )
                            n_mm += 1
                    nc.vector.tensor_scalar_add(
                        convT_sb[:, co, nkb * 512:(nkb + 1) * 512],
                        ps, srb_sb[:, co:co + 1])

            # ---- phase D: kT = k_wT.T @ convT ----
            for kt in range(CT):
                for nkb in range(2):
                    ps = ps1.tile([P, 512], F32)
                    for c in range(CT):
                        nc.tensor.matmul(
                            ps,
                            kw_sb[:, c, kt * 128:(kt + 1) * 128],
                            convT_sb[:, c, nkb * 512:(nkb + 1) * 512],
                            start=(c == 0), stop=(c == CT - 1),
                        )
                    nc.vector.tensor_copy(
                        kTz_sb[0:64, 2 * kt, nkb * 512:(nkb + 1) * 512],
                        ps[0:64, :])
                    nc.vector.tensor_copy(
                        kTz_sb[64:128, 2 * kt + 1, nkb * 512:(nkb + 1) * 512],
                        ps[64:128, :])

            # ---- phase E: v = convT.T @ v_wT (natural layout + ones col) ----
            for nk in range(NKT):
                ps = ps1.tile([P, 512], F32)
                for c in range(CT):
                    nc.tensor.matmul(
                        ps,
                        convT_sb[:, c, nk * 128:(nk + 1) * 128],
                        vw_sb[:, c, :],
                        start=(c == 0), stop=(c == CT - 1),
                    )
                nc.vector.tensor_copy(
                    vaug_sb[:, nk, :, 0:64],
                    ps.rearrange("p (h e) -> p h e", e=64),
                )

        # ---- phase F: attention per head pair, per query half ----
        with ExitStack() as ps_ctx:
            ps_s = ps_ctx.enter_context(
                tc.tile_pool(name="ps_s", bufs=1, space="PSUM"))
            ps_o = ps_ctx.enter_context(
                tc.tile_pool(name="ps_o", bufs=1, space="PSUM"))

            for hf in range(2):
                for pr in range(4):
                    # o[h2][q2]: [65, 512] accumulators (1 PSUM bank each)
                    o_ps = [[ps_o.tile([P, 512], F32, tag=f"o{h2}{q2}",
                                       name=f"o_{pr}_{hf}_{h2}{q2}")
                             for q2 in range(2)] for h2 in range(2)]
                    for nk in range(NKT):
                        s0 = ps_s.tile([P, 1024], F32, tag="s0")
                        s1 = ps_s.tile([P, 1024], F32, tag="s1")
                        for q2 in range(2):
                            nqs = hf * 1024 + q2 * 512
                            nc.tensor.matmul(
                                s0[:, q2 * 512:(q2 + 1) * 512],
                                kTz_sb[:, 2 * pr, nk * 128:(nk + 1) * 128],
                                qT_sb[:, pr, nqs:nqs + 512],
                                start=True, stop=True,
                            )
                            nc.tensor.matmul(
                                s1[:, q2 * 512:(q2 + 1) * 512],
                                kTz_sb[:, 2 * pr + 1, nk * 128:(nk + 1) * 128],
                                qT_sb[:, pr, nqs:nqs + 512],
                                start=True, stop=True,
                            )
                        e0 = expp.tile([P, 1024], BF)
                        e1 = expp.tile([P, 1024], BF)
                        nc.scalar.activation(e0, s0, Exp, scale=SCALE)
                        nc.scalar.activation(e1, s1, Exp, scale=SCALE)
                        for q2 in range(2):
                            qs = q2 * 512
                            nc.tensor.matmul(
                                o_ps[0][q2],
                                vaug_sb[:, nk, 2 * pr, :],
                                e0[:, qs:qs + 512],
                                start=(nk == 0), stop=(nk == NKT - 1),
                            )
                            nc.tensor.matmul(
                                o_ps[1][q2],
                                vaug_sb[:, nk, 2 * pr + 1, :],
                                e1[:, qs:qs + 512],
                                start=(nk == 0), stop=(nk == NKT - 1),
                            )
                    # normalize: OT = O / denom (denom = row 64 of o_ps).
                    # Stage denom rows into SBUF, reciprocal, then per-chunk
                    # broadcast + multiply so each o_ps bank frees asap.
                    d2 = d2p.tile([1, 2048], F32)
                    for h2 in range(2):
                        for q2 in range(2):
                            nc.vector.tensor_copy(
                                d2[0:1, h2 * 1024 + q2 * 512:
                                   h2 * 1024 + (q2 + 1) * 512],
                                o_ps[h2][q2][64:65, :])
                    r2 = d2p.tile([1, 2048], F32)
                    nc.vector.reciprocal_approx_fast(out=r2, in_=d2)
                    for h2 in range(2):
                        for q2 in range(2):
                            rb = rbp.tile([64, 512], F32, tag=f"rb{h2}{q2}",
                                          name=f"rb_{pr}_{hf}_{h2}{q2}")
                            nc.gpsimd.partition_broadcast(
                                rb, r2[0:1, h2 * 1024 + q2 * 512:
                                       h2 * 1024 + (q2 + 1) * 512])
                            hq = hf * 1024 + q2 * 512
                            nc.vector.tensor_mul(
                                oT_sb[h2 * 64:(h2 + 1) * 64, pr, hq:hq + 512],
                                o_ps[h2][q2][0:64, :], rb)


        # ---- phase G: PT = proj_wT.T @ OT + proj_b ----
        with ExitStack() as ps_ctx:
            ps2 = ps_ctx.enter_context(
                tc.tile_pool(name="ps2", bufs=4, space="PSUM"))
            for co in range(CT):
                for nqb in range(4):
                    ps = ps2.tile([P, 512], F32)
                    for c in range(CT):
                        nc.tensor.matmul(
                            ps,
                            pw_sb[:, c, co * 128:(co + 1) * 128],
                            oT_sb[:, c, nqb * 512:(nqb + 1) * 512],
                            start=(c == 0), stop=(c == CT - 1),
                        )
                    pt = outp.tile([P, 512], F32)
                    nc.vector.tensor_scalar_add(pt, ps, pb_sb[:, co:co + 1])
                    dma(out=out_d[co * 128:(co + 1) * 128,
                                  nqb * 512:(nqb + 1) * 512], in_=pt)

    nc.compile()
    return nc


def kernel(x, q_w, kv_w, sr_w, sr_b, proj_w, proj_b, H=64, W=64, **_kw):
    x = np.asarray(x, dtype=np.float32)
    q_w = np.asarray(q_w, dtype=np.float32)
    kv_w = np.asarray(kv_w, dtype=np.float32)
    sr_w = np.asarray(sr_w, dtype=np.float32)
    sr_b = np.asarray(sr_b, dtype=np.float32)
    proj_w = np.asarray(proj_w, dtype=np.float32)
    proj_b = np.asarray(proj_b, dtype=np.float32)
    B, N, C = x.shape

    if "nc" not in _CACHE:
        _CACHE["nc"] = _build_program()
    nc = _CACHE["nc"]

    bf = ml_dtypes.bfloat16
    qw_t = np.ascontiguousarray(q_w.T).astype(bf)              # [c, dq]
    kw_t = np.ascontiguousarray(kv_w[:512].T).astype(bf)       # [c, dk]
    vw_t = np.ascontiguousarray(kv_w[512:].T).astype(bf)       # [c, dv]
    srw_t = np.ascontiguousarray(
        sr_w.transpose(2, 3, 1, 0).reshape(4, 512, 512)).astype(bf)
    pw_t = np.ascontiguousarray(proj_w.T).astype(bf)           # [c, co]

    in_maps = []
    xT = np.ascontiguousarray(x.transpose(0, 2, 1)).astype(bf)  # [B, C, N]
    for c in range(8):
        b, hf = c // 2, c % 2
        in_maps.append({
            "xq": np.ascontiguousarray(xT[b][:, hf * NQ:(hf + 1) * NQ]),
            "xf": xT[b],
            "qw": qw_t, "kw": kw_t, "vw": vw_t,
            "srw": srw_t, "srb": sr_b,
            "pw": pw_t, "pb": proj_b,
        })

    res = run_bass_kernel_spmd(nc, in_maps, core_ids=list(range(8)))
    _CACHE["last_exec_time_ns"] = res.exec_time_ns

    out = np.empty((B, N, C), dtype=np.float32)
    for c in range(8):
        b, hf = c // 2, c % 2
        out[b, hf * NQ:(hf + 1) * NQ, :] = res.results[c]["out_t"].T
    return out


# revision 27
# speedup vs baseline: 1.0295x; 1.0124x over previous
"""Spatial-reduction attention (PVT-style) on 8 Trainium2 NeuronCores.

Shapes (hardcoded): x [4, 4096, 512], 8 heads, head_dim 64, SR=2 conv
reduction -> 1024 keys. Sharding: core c handles batch c//2, query half
c%2 (2048 queries). Conv + kv are recomputed per core pair (cheaper than
a cross-core exchange). All matmul operands bf16, fp32 PSUM accumulate.

Per-core dataflow (everything kept transposed, [channel, token]):
  qT   = q_wT.T @ xq            [512, 2048]
  convT= sum_ij srw_ij.T @ gather_ij(xf) + sr_b   [512, 1024]
  kT   = k_wT.T @ convT         [512, 1024]
  v    = convT.T @ v_wT         [1024, 512]  (natural layout, +ones col)
  ST_h = kT_h.T @ qT_h          [1024, 2048] per head (row-tiled pairs)
  E    = exp(ST * scale)        (ScalarE, bf16 out)
  O_h  = v_aug_h.T @ E          [65, 2048]  (row 64 = softmax denom)
  OT   = O_h / denom            [512, 2048] bf16
  PT   = proj_wT.T @ OT + proj_b  [512, 2048] fp32 -> output (host transposes)
"""

import numpy as np
import ml_dtypes
from contextlib import ExitStack

import concourse.bass as bass
import concourse.mybir as mybir
from concourse import bacc
from concourse.bass_utils import run_bass_kernel_spmd
from concourse.tile import TileContext

BF = mybir.dt.bfloat16
F8 = mybir.dt.float8e4
F32 = mybir.dt.float32
P = 128
CT = 4            # channel tiles (512 / 128)
NQ = 2048         # queries per core
NKT = 8           # key tiles (1024 / 128)
SCALE = 0.125     # 64 ** -0.5

_CACHE = {}


def _build_program():
    nc = bacc.Bacc("TRN2", target_bir_lowering=False, debug=False, num_devices=8)

    xq_d = nc.dram_tensor("xq", [512, NQ], BF, kind="ExternalInput")
    xf_d = nc.dram_tensor("xf", [512, 4096], BF, kind="ExternalInput")
    qw_d = nc.dram_tensor("qw", [512, 512], BF, kind="ExternalInput")      # [c, dq]
    kw_d = nc.dram_tensor("kw", [512, 512], BF, kind="ExternalInput")      # [c, dk]
    vw_d = nc.dram_tensor("vw", [512, 512], BF, kind="ExternalInput")      # [c, dv]
    srw_d = nc.dram_tensor("srw", [4, 512, 512], BF, kind="ExternalInput")  # [ij, ci, co]
    srb_d = nc.dram_tensor("srb", [512], F32, kind="ExternalInput")
    pw_d = nc.dram_tensor("pw", [512, 512], BF, kind="ExternalInput")      # [c, co]
    pb_d = nc.dram_tensor("pb", [512], F32, kind="ExternalInput")
    out_d = nc.dram_tensor("out_t", [512, NQ], F32, kind="ExternalOutput")

    Exp = mybir.ActivationFunctionType.Exp

    with TileContext(nc) as tc, ExitStack() as ctx:
        const = ctx.enter_context(tc.tile_pool(name="const", bufs=1))
        expp = ctx.enter_context(tc.tile_pool(name="expp", bufs=3))
        d2p = ctx.enter_context(tc.tile_pool(name="d2p", bufs=1))
        rbp = ctx.enter_context(tc.tile_pool(name="rbp", bufs=1))
        outp = ctx.enter_context(tc.tile_pool(name="outp", bufs=3))

        dma = nc.sync.dma_start

        # ---- load inputs ----
        qw_sb = const.tile([P, CT, 512], BF)
        qw_r = qw_d.rearrange("(t p) n -> p t n", p=P)
        for t in range(CT):
            dma(out=qw_sb[:, t, :], in_=qw_r[:, t, :])
        xq_sb = const.tile([P, CT, NQ], BF)
        xq_r = xq_d.rearrange("(t p) n -> p t n", p=P)
        for t in range(CT):
            dma(out=xq_sb[:, t, :], in_=xq_r[:, t, :])
        kw_sb = const.tile([P, CT, 512], BF)
        dma(out=kw_sb, in_=kw_d.rearrange("(t p) n -> p t n", p=P))
        vw_sb = const.tile([P, CT, 512], BF)
        dma(out=vw_sb, in_=vw_d.rearrange("(t p) n -> p t n", p=P))
        srw_sb = const.tile([P, 4, CT, 512], BF)
        srw_r = srw_d.rearrange("i (t p) o -> p i t o", p=P)
        for ij4 in range(4):
            dma(out=srw_sb[:, ij4, :, :], in_=srw_r[:, ij4, :, :])
        srb_sb = const.tile([P, CT], F32)
        dma(out=srb_sb, in_=srb_d.rearrange("(t p) -> p t", p=P))
        pw_sb = const.tile([P, CT, 512], BF)
        dma(out=pw_sb, in_=pw_d.rearrange("(t p) n -> p t n", p=P))
        pb_sb = const.tile([P, CT], F32)
        dma(out=pb_sb, in_=pb_d.rearrange("(t p) -> p t", p=P))

        xf_sb = const.tile([P, CT, 4096], BF)
        xf_r = xf_d.rearrange("(t p) n -> p t n", p=P)
        for t in range(CT):
            dma(out=xf_sb[:, t, :], in_=xf_r[:, t, :])

        qT_sb = const.tile([P, CT, NQ], BF)
        convT_sb = const.tile([P, CT, 1024], BF)
        kTz_sb = const.tile([P, 8, 1024], BF)
        vaug_sb = const.tile([P, NKT, 8, 128], BF)
        oT_sb = const.tile([P, CT, NQ], BF)

        nc.gpsimd.memset(vaug_sb, 0.0)
        nc.gpsimd.memset(vaug_sb[:, :, :, 64:65], 1.0)
        nc.gpsimd.memset(kTz_sb, 0.0)

        with ExitStack() as ps_ctx:
            ps1 = ps_ctx.enter_context(tc.tile_pool(name="ps1", bufs=6, space="PSUM"))

            # ---- phase B: qT = q_wT.T @ xq ----
            for dq in range(CT):
                for nqb in range(4):
                    ps = ps1.tile([P, 512], F32)
                    for c in range(CT):
                        nc.tensor.matmul(
                            ps,
                            qw_sb[:, c, dq * 128:(dq + 1) * 128],
                            xq_sb[:, c, nqb * 512:(nqb + 1) * 512],
                            start=(c == 0), stop=(c == CT - 1),
                        )
                    nc.vector.tensor_copy(
                        qT_sb[:, dq, nqb * 512:(nqb + 1) * 512], ps)

            # ---- phase C: convT (spatial reduction) ----
            for co in range(CT):
                for nkb in range(2):
                    ps = ps1.tile([P, 512], F32)
                    n_mm = 0
                    for ij in range(4):
                        i, j = ij >> 1, ij & 1
                        for ci in range(CT):
                            rhs = xf_sb[:, ci, :].rearrange(
                                "p (a i b j) -> p i j a b", a=32, i=2, b=32, j=2
                            )[:, i, j, nkb * 16:(nkb + 1) * 16, :]
                            nc.tensor.matmul(
                                ps,
                                srw_sb[:, ij, ci, co * 128:(co + 1) * 128],
                                rhs,
                                start=(n_mm == 0), stop=(n_mm == 15),
                            )
                            n_mm += 1
                    nc.vector.tensor_scalar_add(
                        convT_sb[:, co, nkb * 512:(nkb + 1) * 512],
                        ps, srb_sb[:, co:co + 1])

            # ---- phase D: kT = k_wT.T @ convT ----
            for kt in range(CT):
                for nkb in range(2):
                    ps = ps1.tile([P, 512], F32)
                    for c in range(CT):
                        nc.tensor.matmul(
                            ps,
                            kw_sb[:, c, kt * 128:(kt + 1) * 128],
                            convT_sb[:, c, nkb * 512:(nkb + 1) * 512],
                            start=(c == 0), stop=(c == CT - 1),
                        )
                    nc.vector.tensor_copy(
                        kTz_sb[0:64, 2 * kt, nkb * 512:(nkb + 1) * 512],
                        ps[0:64, :])
                    nc.vector.tensor_copy(
                        kTz_sb[64:128, 2 * kt + 1, nkb * 512:(nkb + 1) * 512],
                        ps[64:128, :])

            # ---- phase E: v = convT.T @ v_wT (natural layout + ones col) ----
            for nk in range(NKT):
                ps = ps1.tile([P, 512], F32)
                for c in range(CT):
                    nc.tensor.matmul(
                        ps,
                        convT_sb[:, c, nk * 128:(nk + 1) * 128],
                        vw_sb[:, c, :],
                        start=(c == 0), stop=(c == CT - 1),
                    )
                nc.vector.tensor_copy(
                    vaug_sb[:, nk, :, 0:64],
                    ps.rearrange("p (h e) -> p h e", e=64),
                )

        # ---- phase F: attention per head pair, per query half ----
        with ExitStack() as ps_ctx:
            ps_s = ps_ctx.enter_context(
                tc.tile_pool(name="ps_s", bufs=1, space="PSUM"))
            ps_o = ps_ctx.enter_context(
                tc.tile_pool(name="ps_o", bufs=1, space="PSUM"))

            for hf in range(2):
                for pr in range(4):
                    # o[h2][q2]: [65, 512] accumulators (1 PSUM bank each)
                    o_ps = [[ps_o.tile([P, 512], F32, tag=f"o{h2}{q2}",
                                       name=f"o_{pr}_{hf}_{h2}{q2}")
                             for q2 in range(2)] for h2 in range(2)]
                    for nk in range(NKT):
                        s0 = ps_s.tile([P, 1024], F32, tag="s0")
                        s1 = ps_s.tile([P, 1024], F32, tag="s1")
                        for q2 in range(2):
                            nqs = hf * 1024 + q2 * 512
                            nc.tensor.matmul(
                                s0[:, q2 * 512:(q2 + 1) * 512],
                                kTz_sb[:, 2 * pr, nk * 128:(nk + 1) * 128],
                                qT_sb[:, pr, nqs:nqs + 512],
                                start=True, stop=True,
                            )
                            nc.tensor.matmul(
                                s1[:, q2 * 512:(q2 + 1) * 512],
                                kTz_sb[:, 2 * pr + 1, nk * 128:(nk + 1) * 128],
                                qT_sb[:, pr, nqs:nqs + 512],
                                start=True, stop=True,
                            )
                        e0 = expp.tile([P, 1024], BF)
                        e1 = expp.tile([P, 1024], BF)
                        nc.scalar.activation(e0, s0, Exp, scale=SCALE)
                        nc.scalar.activation(e1, s1, Exp, scale=SCALE)
                        for q2 in range(2):
                            qs = q2 * 512
                            nc.tensor.matmul(
                                o_ps[0][q2],
                                vaug_sb[:, nk, 2 * pr, :],
                                e0[:, qs:qs + 512],
                                start=(nk == 0), stop=(nk == NKT - 1),
                            )
                            nc.tensor.matmul(
                                o_ps[1][q2],
                                vaug_sb[:, nk, 2 * pr + 1, :],
                                e1[:, qs:qs + 512],
                                start=(nk == 0), stop=(nk == NKT - 1),
                            )
                    # normalize: OT = O / denom (denom = row 64 of o_ps).
                    # Stage denom rows into SBUF, reciprocal, then per-chunk
                    # broadcast + multiply so each o_ps bank frees asap.
                    d2 = d2p.tile([1, 2048], F32)
                    for h2 in range(2):
                        for q2 in range(2):
                            nc.vector.tensor_copy(
                                d2[0:1, h2 * 1024 + q2 * 512:
                                   h2 * 1024 + (q2 + 1) * 512],
                                o_ps[h2][q2][64:65, :])
                    r2 = d2p.tile([1, 2048], F32)
                    nc.vector.reciprocal_approx_fast(out=r2, in_=d2)
                    for h2 in range(2):
                        for q2 in range(2):
                            rb = rbp.tile([64, 512], F32, tag=f"rb{h2}{q2}",
                                          name=f"rb_{pr}_{hf}_{h2}{q2}")
                            nc.gpsimd.partition_broadcast(
                                rb, r2[0:1, h2 * 1024 + q2 * 512:
                                       h2 * 1024 + (q2 + 1) * 512])
                            hq = hf * 1024 + q2 * 512
                            nc.vector.tensor_mul(
                                oT_sb[h2 * 64:(h2 + 1) * 64, pr, hq:hq + 512],
                                o_ps[h2][q2][0:64, :], rb)


        # ---- phase G: PT = proj_wT.T @ OT + proj_b ----
        with ExitStack() as ps_ctx:
            ps2 = ps_ctx.enter_context(
                tc.tile_pool(name="ps2", bufs=4, space="PSUM"))
            for co in range(CT):
                for nqb in range(4):
                    ps = ps2.tile([P, 512], F32)
                    for c in range(CT):
                        nc.tensor.matmul(
                            ps,
                            pw_sb[:, c, co * 128:(co + 1) * 128],
                            oT_sb[:, c, nqb * 512:(nqb + 1) * 512],
                            start=(c == 0), stop=(c == CT - 1),
                        )
                    pt = outp.tile([P, 512], F32)
                    nc.vector.tensor_scalar_add(pt, ps, pb_sb[:, co:co + 1])
                    dma(out=out_d[co * 128:(co + 1) * 128,
                                  nqb * 512:(nqb + 1) * 512], in_=pt)

    nc.compile()
    return nc


def kernel(x, q_w, kv_w, sr_w, sr_b, proj_w, proj_b, H=64, W=64, **_kw):
    x = np.asarray(x, dtype=np.float32)
    q_w = np.asarray(q_w, dtype=np.float32)
    kv_w = np.asarray(kv_w, dtype=np.float32)
    sr_w = np.asarray(sr_w, dtype=np.float32)
    sr_b = np.asarray(sr_b, dtype=np.float32)
    proj_w = np.asarray(proj_w, dtype=np.float32)
    proj_b = np.asarray(proj_b, dtype=np.float32)
    B, N, C = x.shape

    if "nc" not in _CACHE:
        _CACHE["nc"] = _build_program()
    nc = _CACHE["nc"]

    bf = ml_dtypes.bfloat16
    qw_t = np.ascontiguousarray(q_w.T).astype(bf)              # [c, dq]
    kw_t = np.ascontiguousarray(kv_w[:512].T).astype(bf)       # [c, dk]
    vw_t = np.ascontiguousarray(kv_w[512:].T).astype(bf)       # [c, dv]
    srw_t = np.ascontiguousarray(
        sr_w.transpose(2, 3, 1, 0).reshape(4, 512, 512)).astype(bf)
    pw_t = np.ascontiguousarray(proj_w.T).astype(bf)           # [c, co]

    in_maps = []
    xT = np.ascontiguousarray(x.transpose(0, 2, 1)).astype(bf)  # [B, C, N]
    for c in range(8):
        b, hf = c // 2, c % 2
        in_maps.append({
            "xq": np.ascontiguousarray(xT[b][:, hf * NQ:(hf + 1) * NQ]),
            "xf": xT[b],
            "qw": qw_t, "kw": kw_t, "vw": vw_t,
            "srw": srw_t, "srb": sr_b,
            "pw": pw_t, "pb": proj_b,
        })

    res = run_bass_kernel_spmd(nc, in_maps, core_ids=list(range(8)))
    _CACHE["last_exec_time_ns"] = res.exec_time_ns

    out = np.empty((B, N, C), dtype=np.float32)
    for c in range(8):
        b, hf = c // 2, c % 2
        out[b, hf * NQ:(hf + 1) * NQ, :] = res.results[c]["out_t"].T
    return out


# revision 30
# speedup vs baseline: 1.0317x; 1.0021x over previous
"""Spatial-reduction attention (PVT-style) on 8 Trainium2 NeuronCores.

Shapes (hardcoded): x [4, 4096, 512], 8 heads, head_dim 64, SR=2 conv
reduction -> 1024 keys. Sharding: core c handles batch c//2, query half
c%2 (2048 queries). Conv + kv are recomputed per core pair (cheaper than
a cross-core exchange). All matmul operands bf16, fp32 PSUM accumulate.

Per-core dataflow (everything kept transposed, [channel, token]):
  qT   = q_wT.T @ xq            [512, 2048]
  convT= sum_ij srw_ij.T @ gather_ij(xf) + sr_b   [512, 1024]
  kT   = k_wT.T @ convT         [512, 1024]
  v    = convT.T @ v_wT         [1024, 512]  (natural layout, +ones col)
  ST_h = kT_h.T @ qT_h          [1024, 2048] per head (row-tiled pairs)
  E    = exp(ST * scale)        (ScalarE, bf16 out)
  O_h  = v_aug_h.T @ E          [65, 2048]  (row 64 = softmax denom)
  OT   = O_h / denom            [512, 2048] bf16
  PT   = proj_wT.T @ OT + proj_b  [512, 2048] fp32 -> output (host transposes)
"""

import numpy as np
import ml_dtypes
from contextlib import ExitStack

import concourse.bass as bass
import concourse.mybir as mybir
from concourse import bacc
from concourse.bass_utils import run_bass_kernel_spmd
from concourse.tile import TileContext

BF = mybir.dt.bfloat16
F8 = mybir.dt.float8e4
F32 = mybir.dt.float32
P = 128
CT = 4            # channel tiles (512 / 128)
NQ = 2048         # queries per core
NKT = 8           # key tiles (1024 / 128)
SCALE = 0.125     # 64 ** -0.5

_CACHE = {}


def _build_program():
    nc = bacc.Bacc("TRN2", target_bir_lowering=False, debug=False, num_devices=8)

    xq_d = nc.dram_tensor("xq", [512, NQ], BF, kind="ExternalInput")
    xf_d = nc.dram_tensor("xf", [512, 4096], BF, kind="ExternalInput")
    qw_d = nc.dram_tensor("qw", [512, 512], BF, kind="ExternalInput")      # [c, dq]
    kw_d = nc.dram_tensor("kw", [512, 512], BF, kind="ExternalInput")      # [c, dk]
    vw_d = nc.dram_tensor("vw", [512, 512], BF, kind="ExternalInput")      # [c, dv]
    srw_d = nc.dram_tensor("srw", [4, 512, 512], BF, kind="ExternalInput")  # [ij, ci, co]
    srb_d = nc.dram_tensor("srb", [512], F32, kind="ExternalInput")
    pw_d = nc.dram_tensor("pw", [512, 512], BF, kind="ExternalInput")      # [c, co]
    pb_d = nc.dram_tensor("pb", [512], F32, kind="ExternalInput")
    out_d = nc.dram_tensor("out_t", [512, NQ], F32, kind="ExternalOutput")

    Exp = mybir.ActivationFunctionType.Exp

    with TileContext(nc) as tc, ExitStack() as ctx:
        const = ctx.enter_context(tc.tile_pool(name="const", bufs=1))
        expp = ctx.enter_context(tc.tile_pool(name="expp", bufs=3))
        d2p = ctx.enter_context(tc.tile_pool(name="d2p", bufs=1))
        rbp = ctx.enter_context(tc.tile_pool(name="rbp", bufs=1))
        outp = ctx.enter_context(tc.tile_pool(name="outp", bufs=3))

        dma = nc.sync.dma_start

        # ---- load inputs ----
        qw_sb = const.tile([P, CT, 512], BF)
        qw_r = qw_d.rearrange("(t p) n -> p t n", p=P)
        for t in range(CT):
            dma(out=qw_sb[:, t, :], in_=qw_r[:, t, :])
        xq_sb = const.tile([P, CT, NQ], BF)
        xq_r = xq_d.rearrange("(t p) n -> p t n", p=P)
        for t in range(CT):
            dma(out=xq_sb[:, t, :], in_=xq_r[:, t, :])
        kw_sb = const.tile([P, CT, 512], BF)
        dma(out=kw_sb, in_=kw_d.rearrange("(t p) n -> p t n", p=P))
        vw_sb = const.tile([P, CT, 512], BF)
        dma(out=vw_sb, in_=vw_d.rearrange("(t p) n -> p t n", p=P))
        srw_sb = const.tile([P, 4, CT, 512], BF)
        srw_r = srw_d.rearrange("i (t p) o -> p i t o", p=P)
        for ij4 in range(4):
            dma(out=srw_sb[:, ij4, :, :], in_=srw_r[:, ij4, :, :])
        srb_sb = const.tile([P, CT], F32)
        dma(out=srb_sb, in_=srb_d.rearrange("(t p) -> p t", p=P))
        pw_sb = const.tile([P, CT, 512], BF)
        dma(out=pw_sb, in_=pw_d.rearrange("(t p) n -> p t n", p=P))
        pb_sb = const.tile([P, CT], F32)
        dma(out=pb_sb, in_=pb_d.rearrange("(t p) -> p t", p=P))

        xf_sb = const.tile([P, CT, 4096], BF)
        xf_r = xf_d.rearrange("(t p) n -> p t n", p=P)
        for t in range(CT):
            dma(out=xf_sb[:, t, :], in_=xf_r[:, t, :])

        qT_sb = const.tile([P, CT, NQ], BF)
        convT_sb = const.tile([P, CT, 1024], BF)
        kTz_sb = const.tile([P, 8, 1024], BF)
        vaug_sb = const.tile([P, NKT, 8, 128], BF)
        oT_sb = const.tile([P, CT, NQ], BF)

        nc.gpsimd.memset(vaug_sb, 0.0)
        nc.gpsimd.memset(vaug_sb[:, :, :, 64:65], 1.0)
        nc.gpsimd.memset(kTz_sb, 0.0)

        with ExitStack() as ps_ctx:
            ps1 = ps_ctx.enter_context(tc.tile_pool(name="ps1", bufs=6, space="PSUM"))

            # ---- phase B: qT = q_wT.T @ xq ----
            for dq in range(CT):
                for nqb in range(4):
                    ps = ps1.tile([P, 512], F32)
                    for c in range(CT):
                        nc.tensor.matmul(
                            ps,
                            qw_sb[:, c, dq * 128:(dq + 1) * 128],
                            xq_sb[:, c, nqb * 512:(nqb + 1) * 512],
                            start=(c == 0), stop=(c == CT - 1),
                        )
                    nc.vector.tensor_copy(
                        qT_sb[:, dq, nqb * 512:(nqb + 1) * 512], ps)

            # ---- phase C: convT (spatial reduction) ----
            for co in range(CT):
                for nkb in range(2):
                    ps = ps1.tile([P, 512], F32)
                    n_mm = 0
                    for ij in range(4):
                        i, j = ij >> 1, ij & 1
                        for ci in range(CT):
                            rhs = xf_sb[:, ci, :].rearrange(
                                "p (a i b j) -> p i j a b", a=32, i=2, b=32, j=2
                            )[:, i, j, nkb * 16:(nkb + 1) * 16, :]
                            nc.tensor.matmul(
                                ps,
                                srw_sb[:, ij, ci, co * 128:(co + 1) * 128],
                                rhs,
                                start=(n_mm == 0), stop=(n_mm == 15),
                            )
                            n_mm += 1
                    nc.vector.tensor_scalar_add(
                        convT_sb[:, co, nkb * 512:(nkb + 1) * 512],
                        ps, srb_sb[:, co:co + 1])

            # ---- phase D: kT = k_wT.T @ convT ----
            for kt in range(CT):
                for nkb in range(2):
                    ps = ps1.tile([P, 512], F32)
                    for c in range(CT):
                        nc.tensor.matmul(
                            ps,
                            kw_sb[:, c, kt * 128:(kt + 1) * 128],
                            convT_sb[:, c, nkb * 512:(nkb + 1) * 512],
                            start=(c == 0), stop=(c == CT - 1),
                        )
                    nc.vector.tensor_copy(
                        kTz_sb[0:64, 2 * kt, nkb * 512:(nkb + 1) * 512],
                        ps[0:64, :])
                    nc.vector.tensor_copy(
                        kTz_sb[64:128, 2 * kt + 1, nkb * 512:(nkb + 1) * 512],
                        ps[64:128, :])

            # ---- phase E: v = convT.T @ v_wT (natural layout + ones col) ----
            for nk in range(NKT):
                ps = ps1.tile([P, 512], F32)
                for c in range(CT):
                    nc.tensor.matmul(
                        ps,
                        convT_sb[:, c, nk * 128:(nk + 1) * 128],
                        vw_sb[:, c, :],
                        start=(c == 0), stop=(c == CT - 1),
                    )
                nc.vector.tensor_copy(
                    vaug_sb[:, nk, :, 0:64],
                    ps.rearrange("p (h e) -> p h e", e=64),
                )

        # ---- phase F: attention per head pair, per query half ----
        with ExitStack() as ps_ctx:
            ps_s = ps_ctx.enter_context(
                tc.tile_pool(name="ps_s", bufs=1, space="PSUM"))
            ps_o = ps_ctx.enter_context(
                tc.tile_pool(name="ps_o", bufs=1, space="PSUM"))

            for hf in range(2):
                for pr in range(4):
                    # o[h2][q2]: [65, 512] accumulators (1 PSUM bank each)
                    o_ps = [[ps_o.tile([P, 512], F32, tag=f"o{h2}{q2}",
                                       name=f"o_{pr}_{hf}_{h2}{q2}")
                             for q2 in range(2)] for h2 in range(2)]
                    for nk in range(NKT):
                        s0 = ps_s.tile([P, 1024], F32, tag="s0")
                        s1 = ps_s.tile([P, 1024], F32, tag="s1")
                        for q2 in range(2):
                            nqs = hf * 1024 + q2 * 512
                            nc.tensor.matmul(
                                s0[:, q2 * 512:(q2 + 1) * 512],
                                kTz_sb[:, 2 * pr, nk * 128:(nk + 1) * 128],
                                qT_sb[:, pr, nqs:nqs + 512],
                                start=True, stop=True,
                            )
                            nc.tensor.matmul(
                                s1[:, q2 * 512:(q2 + 1) * 512],
                                kTz_sb[:, 2 * pr + 1, nk * 128:(nk + 1) * 128],
                                qT_sb[:, pr, nqs:nqs + 512],
                                start=True, stop=True,
                            )
                        e0 = expp.tile([P, 1024], BF)
                        e1 = expp.tile([P, 1024], BF)
                        nc.scalar.activation(e0, s0, Exp, scale=SCALE)
                        nc.scalar.activation(e1, s1, Exp, scale=SCALE)
                        for q2 in range(2):
                            qs = q2 * 512
                            nc.tensor.matmul(
                                o_ps[0][q2],
                                vaug_sb[:, nk, 2 * pr, :],
                                e0[:, qs:qs + 512],
                                start=(nk == 0), stop=(nk == NKT - 1),
                            )
                            nc.tensor.matmul(
                                o_ps[1][q2],
                                vaug_sb[:, nk, 2 * pr + 1, :],
                                e1[:, qs:qs + 512],
                                start=(nk == 0), stop=(nk == NKT - 1),
                            )
                    # normalize: OT = O / denom (denom = row 64 of o_ps).
                    # Stage denom rows into SBUF, reciprocal, then per-chunk
                    # broadcast + multiply so each o_ps bank frees asap.
                    d2 = d2p.tile([1, 2048], F32)
                    for h2 in range(2):
                        for q2 in range(2):
                            nc.vector.tensor_copy(
                                d2[0:1, h2 * 1024 + q2 * 512:
                                   h2 * 1024 + (q2 + 1) * 512],
                                o_ps[h2][q2][64:65, :])
                    r2 = d2p.tile([1, 2048], F32)
                    nc.vector.reciprocal_approx_fast(out=r2, in_=d2)
                    for h2 in range(2):
                        for q2 in range(2):
                            rb = rbp.tile([64, 512], F32, tag=f"rb{h2}{q2}",
                                          name=f"rb_{pr}_{hf}_{h2}{q2}")
                            nc.gpsimd.partition_broadcast(
                                rb, r2[0:1, h2 * 1024 + q2 * 512:
                                       h2 * 1024 + (q2 + 1) * 512])
                            hq = hf * 1024 + q2 * 512
                            nc.vector.tensor_mul(
                                oT_sb[h2 * 64:(h2 + 1) * 64, pr, hq:hq + 512],
                                o_ps[h2][q2][0:64, :], rb)


        # ---- phase G: PT = proj_wT.T @ OT + proj_b ----
        with ExitStack() as ps_ctx:
            ps2 = ps_ctx.enter_context(
                tc.tile_pool(name="ps2", bufs=4, space="PSUM"))
            for co in range(CT):
                for nqb in range(4):
                    ps = ps2.tile([P, 512], F32)
                    for c in range(CT):
                        nc.tensor.matmul(
                            ps,
                            pw_sb[:, c, co * 128:(co + 1) * 128],
                            oT_sb[:, c, nqb * 512:(nqb + 1) * 512],
                            start=(c == 0), stop=(c == CT - 1),
                        )
                    pt = outp.tile([P, 512], F32)
                    nc.vector.tensor_scalar_add(pt, ps, pb_sb[:, co:co + 1])
                    dma(out=out_d[co * 128:(co + 1) * 128,
                                  nqb * 512:(nqb + 1) * 512], in_=pt)

    nc.compile()
    return nc


def kernel(x, q_w, kv_w, sr_w, sr_b, proj_w, proj_b, H=64, W=64, **_kw):
    x = np.asarray(x, dtype=np.float32)
    q_w = np.asarray(q_w, dtype=np.float32)
    kv_w = np.asarray(kv_w, dtype=np.float32)
    sr_w = np.asarray(sr_w, dtype=np.float32)
    sr_b = np.asarray(sr_b, dtype=np.float32)
    proj_w = np.asarray(proj_w, dtype=np.float32)
    proj_b = np.asarray(proj_b, dtype=np.float32)
    B, N, C = x.shape

    if "nc" not in _CACHE:
        _CACHE["nc"] = _build_program()
    nc = _CACHE["nc"]

    bf = ml_dtypes.bfloat16
    qw_t = np.ascontiguousarray(q_w.T).astype(bf)              # [c, dq]
    kw_t = np.ascontiguousarray(kv_w[:512].T).astype(bf)       # [c, dk]
    vw_t = np.ascontiguousarray(kv_w[512:].T).astype(bf)       # [c, dv]
    srw_t = np.ascontiguousarray(
        sr_w.transpose(2, 3, 1, 0).reshape(4, 512, 512)).astype(bf)
    pw_t = np.ascontiguousarray(proj_w.T).astype(bf)           # [c, co]

    in_maps = []
    xT = np.ascontiguousarray(x.transpose(0, 2, 1)).astype(bf)  # [B, C, N]
    for c in range(8):
        b, hf = c // 2, c % 2
        in_maps.append({
            "xq": np.ascontiguousarray(xT[b][:, hf * NQ:(hf + 1) * NQ]),
            "xf": xT[b],
            "qw": qw_t, "kw": kw_t, "vw": vw_t,
            "srw": srw_t, "srb": sr_b,
            "pw": pw_t, "pb": proj_b,
        })

    res = run_bass_kernel_spmd(nc, in_maps, core_ids=list(range(8)))
    _CACHE["last_exec_time_ns"] = res.exec_time_ns

    out = np.empty((B, N, C), dtype=np.float32)
    for c in range(8):
        b, hf = c // 2, c % 2
        out[b, hf * NQ:(hf + 1) * NQ, :] = res.results[c]["out_t"].T
    return out
